# revision 4
# baseline (speedup 1.0000x reference)
"""Trainium2 Bass kernel v2: multi-head attention (B=4, N=2048, C=1024, H=16).

Sharding: 8 cores = 4 batches x 2 head-halves (tensor parallel over heads).
Each core computes q/k/v for its 8 heads over the full 2048 rows of its
batch, attention for those heads, and a per-head-half partial output
projection y_part = O_norm @ Wout[head-slice]; the host sums the two
partials per batch (plus bout). No K/V duplication, no collectives.

Per head-pair the normalized O lives in SBUF; the output projection is
emitted per (pair, query-chunk) as soon as that slice is normalized, so
Y streams through the attention phase instead of a serial tail.

exp tiles are split between the Scalar engine (native Exp, bias -ln8
folded) and the Vector engine (Schraudolph fast-exp: int16 = S*128/ln2
+ B bitcast as bf16, ~1.8% rms on 6/16 tiles; fp8 P/V + DoubleRow AV
was tried and rejected - with random data softmax averaging shrinks
signal and quantization noise equally, so e4m3's ~3.6% per-element
error passes through to the output). Row sums ride the AV matmul via
a ones column in V. PSUM egress is ACT/DVE only (GPSIMD cannot touch
PSUM); GpSimd handles the SBUF-side normalization multiplies and
issues the bulk xT input DMAs to keep the SP queue free. The per-pair
output projection is emitted per query-chunk right after that slice's
rowsum reciprocals land, so Y streams through the attention phase;
Y of the last six (pair, chunk) slices is deferred into the tail so
the PE stays busy while the final rowsum->reciprocal DMA chains
complete.
"""

import numpy as np
import ml_dtypes

B, N, C, H = 4, 2048, 1024, 16
DH = C // H                      # 64
SCALE = DH ** -0.5
NCORES = 8
HC = 8                           # heads per core
PR = 4                           # head pairs per core
CT = C // 128                    # 8 contraction tiles
MT = N // 128                    # 16 key tiles
MPAIR = MT // 2                  # 8 key-tile pairs (DoubleRow)
NQC = 4                          # query chunks of 512
NU = PR * NQC                    # 16 attention units

LNK = float(np.log(8.0))         # fold exp(S - ln 8) so P' = exp(S)/8
# DVE fast-exp (Schraudolph in bf16 bit-space, rint convert, HW verified):
# int16 = S*(128/ln2) + (127*128 - 3*128 + delta); bitcast as bf16.
A_SCH = 128.0 / np.log(2.0)
B_SCH = 15864.55                 # rint calibration, min weighted softmax err

AV_FP8 = False                   # fp8 P/V costs ~3% output err here: bf16 AV
DVE_EXP_MTS = (1, 4, 6, 9, 11, 14)  # which mt of each unit exp on DVE
PTB = 1                          # pt bank count

_BF16 = ml_dtypes.bfloat16
_cache = {}


def _patch_tile_drain():
    """Walrus in this env rejects >1 sem wait per instruction; split the tail
    Drain's waits into standalone single-wait nops on SP."""
    import concourse.tile as tile
    import concourse.mybir as mybir
    from concourse.vector_clock import ScopedClock

    if getattr(tile.TileContext, "_drain_split_patched", False):
        return

    def _patched(self, tick_clock, wait_clock):
        nc = self.nc
        drain_inst = nc.sync.drain()
        wait_clock.add_sem_waits(
            drain_inst.ins, ScopedClock({None: tick_clock.global_clock})
        )
        si = drain_inst.ins.sync_info
        waits = list(si.on_wait) if si is not None and si.on_wait else []
        if len(waits) > 1:
            si.on_wait = []
            for w in waits:
                nop = nc.sync.nop(hint="drain_wait_split", nofuse=True)
                nsi = nop.ins.sync_info
                if nsi is None:
                    nop.ins.sync_info = mybir.SyncInfo(on_wait=[w], on_update=[])
                else:
                    nsi.on_wait = [w]
        nc.all_engine_barrier()
        assert self.sems is not None
        popped = nc._tile_sem_poison_stack.pop()
        assert popped is self._sem_poison
        nc.clear_and_free_semaphores(list(self.sems.allocated().values()))
        nc.all_engine_barrier()

    tile.TileContext._drain_and_barrier = _patched
    tile.TileContext._drain_split_patched = True


def _split_excess_waits(nc, limit=1):
    """Walrus here rejects instructions carrying more than `limit` sem waits.
    Move the excess onto same-engine nops inserted immediately before."""
    import concourse.mybir as mybir

    counter = [0]
    for block in nc.m.functions[0].blocks:
        il = block.instructions
        i = 0
        while i < len(il):
            inst = il[i]
            si = inst.sync_info
            waits = list(si.on_wait) if si is not None and si.on_wait else []
            if len(waits) > limit:
                keep = waits[-limit:]
                extra = waits[:-limit]
                si.on_wait = keep
                pos = i
                for j in range(0, len(extra), limit):
                    chunk = extra[j : j + limit]
                    counter[0] += 1
                    nop = mybir.InstNoOp(
                        name=f"waitsplit_{counter[0]}",
                        engine=inst.engine,
                        ins=[],
                        outs=[],
                        sync_info=mybir.SyncInfo(on_wait=chunk, on_update=[]),
                    )
                    try:
                        nc.register_instruction(nop, overwrite=True)
                    except Exception:
                        pass
                    il.insert(pos, nop)
                    pos += 1
                    i += 1
            i += 1


def build_nc():
    import concourse.bass as bass
    import concourse.mybir as mybir
    import concourse.tile as tile

    _patch_tile_drain()
    f32 = mybir.dt.float32
    bf16 = mybir.dt.bfloat16
    fp8 = mybir.dt.float8e4
    u8 = mybir.dt.uint8
    i16 = mybir.dt.int16
    EXP = mybir.ActivationFunctionType.Exp
    MUL = mybir.AluOpType.mult
    ADD = mybir.AluOpType.add
    DR = mybir.MatmulPerfMode.DoubleRow

    pdt = fp8 if AV_FP8 else bf16

    nc = bass.Bass("TRN2", num_devices=NCORES)
    xT = nc.dram_tensor("xT", [C, N], bf16, kind="ExternalInput")
    Wq = nc.dram_tensor("Wq", [C, 512], bf16, kind="ExternalInput")
    Wk = nc.dram_tensor("Wk", [C, 512], bf16, kind="ExternalInput")
    Wv = nc.dram_tensor("Wv", [C, 512], bf16, kind="ExternalInput")
    Wout = nc.dram_tensor("Wout", [512, C], bf16, kind="ExternalInput")
    yp = nc.dram_tensor("yp", [PR, N, C], bf16, kind="ExternalOutput")

    xT_r = xT.ap().rearrange("(a p) n -> p a n", p=128)
    Wq_r = Wq.ap().rearrange("(a p) d -> p a d", p=128)
    Wk_r = Wk.ap().rearrange("(a p) d -> p a d", p=128)
    Wv_r = Wv.ap().rearrange("(a p) d -> p a d", p=128)
    Wout_r = Wout.ap().rearrange("(a p) d -> p a d", p=128)

    with tile.TileContext(nc) as tc:
      with (
          tc.tile_pool(name="persist", bufs=1) as persist,
          tc.tile_pool(name="small", bufs=2) as small,
          tc.tile_pool(name="ys_pool", bufs=4) as ys_pool,
          tc.tile_pool(name="dramp", bufs=1, space="DRAM") as dram_pool,
          tc.tile_pool(name="ps_st", bufs=2, space="PSUM") as ps_st,
          tc.tile_pool(name="ps_ot", bufs=1, space="PSUM") as ps_ot,
          tc.tile_pool(name="ps_pp", bufs=2, space="PSUM") as ps_pp,
      ):
        xT_t = persist.tile([128, CT, N], bf16, name="xT_t")
        Wq_t = persist.tile([128, CT, 512], bf16, name="Wq_t")
        Wk_t = persist.tile([128, CT, 512], bf16, name="Wk_t")
        Wv_t = persist.tile([128, CT, 512], bf16, name="Wv_t")
        Wout_t = persist.tile([128, PR, C], bf16, name="Wout_t")
        qT_sb = [persist.tile([128, N], bf16, name=f"qT{p}") for p in range(PR)]
        kT_sb = [persist.tile([128, N], bf16, name=f"kT{p}") for p in range(PR)]
        if AV_FP8:
            # dual-fp8 ldweights needs a power-of-2 plane stride: pad to 128
            v8 = persist.tile([128, MPAIR, HC, 2, 128], pdt, name="v8")
        else:
            v8 = persist.tile([128, MT, HC, DH + 1], pdt, name="v8")
        pt = [
            persist.tile([128, MPAIR, 2, 1024], pdt, name=f"pt{i}")
            for i in range(PTB)
        ]
        OT = [persist.tile([128, N], bf16, name=f"OT{p}") for p in range(PR)]
        bias_exp = persist.tile([128, 1], f32, name="bias_exp")
        rinv_dram = dram_pool.tile([HC, N], f32)
        rs_dram = dram_pool.tile([HC, N], f32)

        # ---- DMA kickoff (chunked so compute starts early) ----
        for jc in range(CT):
            eng = nc.sync if jc == 0 else nc.gpsimd
            eng.dma_start(out=xT_t[:, jc, :], in_=xT_r[:, jc, :])
            nc.sync.dma_start(out=Wq_t[:, jc, :], in_=Wq_r[:, jc, :])
            nc.sync.dma_start(out=Wk_t[:, jc, :], in_=Wk_r[:, jc, :])
        for jc in range(CT):
            nc.sync.dma_start(out=Wv_t[:, jc, :], in_=Wv_r[:, jc, :])
        nc.sync.dma_start(out=Wout_t, in_=Wout_r)
        nc.vector.memset(bias_exp, -LNK)
        if AV_FP8:
            nc.vector.memset(v8[:, :, :, :, DH : DH + 1], 1.0)
        else:
            nc.vector.memset(v8[:, :, :, DH : DH + 1], 1.0)

        # ---- projection emitters ----
        def emit_q(p, ch):
            ps = ps_pp.tile([128, 512], f32, tag="pp", name=f"psq{p}_{ch}")
            for jc in range(CT):
                nc.tensor.matmul(
                    ps,
                    Wq_t[:, jc, p * 128 : (p + 1) * 128],
                    xT_t[:, jc, ch * 512 : (ch + 1) * 512],
                    start=(jc == 0),
                    stop=(jc == CT - 1),
                )
            nc.vector.tensor_copy(
                out=qT_sb[p][:, ch * 512 : (ch + 1) * 512], in_=ps
            )

        def emit_k(p, ch):
            ps = ps_pp.tile([128, 512], f32, tag="pp", name=f"psk{p}_{ch}")
            for jc in range(CT):
                nc.tensor.matmul(
                    ps,
                    Wk_t[:, jc, p * 128 : (p + 1) * 128],
                    xT_t[:, jc, ch * 512 : (ch + 1) * 512],
                    start=(jc == 0),
                    stop=(jc == CT - 1),
                )
            nc.vector.tensor_copy(
                out=kT_sb[p][:, ch * 512 : (ch + 1) * 512], in_=ps
            )

        def emit_v(mt):
            ps = ps_pp.tile([128, 512], f32, tag="pp", name=f"psv{mt}")
            for jc in range(CT):
                nc.tensor.matmul(
                    ps,
                    xT_t[:, jc, mt * 128 : (mt + 1) * 128],
                    Wv_t[:, jc, :],
                    start=(jc == 0),
                    stop=(jc == CT - 1),
                )
            if AV_FP8:
                dst = v8[:, mt // 2, :, mt % 2, 0:DH]
            else:
                dst = v8[:, mt, :, 0:DH]
            nc.vector.tensor_copy(
                out=dst, in_=ps.rearrange("p (h d) -> p h d", h=HC)
            )

        # ---- attention unit pieces ----
        def emit_s(u, mt):
            p, qc = u // NQC, u % NQC
            ms = slice(mt * 128, (mt + 1) * 128)
            qs = slice(qc * 512, (qc + 1) * 512)
            st = ps_st.tile([128, 1024], f32, tag="st", name=f"st{u}_{mt}")
            nc.tensor.matmul(
                st[:, 0:512], kT_sb[p][0:64, ms], qT_sb[p][0:64, qs],
                start=True, stop=True, tile_position=(0, 0),
            )
            nc.tensor.matmul(
                st[:, 512:1024], kT_sb[p][64:128, ms], qT_sb[p][64:128, qs],
                start=True, stop=True, tile_position=(64, 0),
            )
            return st

        def emit_exp(u, mt, st):
            bank = pt[u % PTB]
            if AV_FP8:
                dst = bank[:, mt // 2, mt % 2, :]
            else:
                dst = bank[:, mt // 2, mt % 2, :]
            if mt in DVE_EXP_MTS:
                cast_dt = u8 if AV_FP8 else i16
                nc.vector.tensor_scalar(
                    out=dst.bitcast(cast_dt), in0=st,
                    scalar1=A_SCH, scalar2=B_SCH, op0=MUL, op1=ADD,
                )
            else:
                nc.scalar.activation(out=dst, in_=st, func=EXP, bias=bias_exp)

        def emit_av(u, mp, ot_pair):
            p = u // NQC
            bank = pt[u % PTB]
            for h in range(2):
                if AV_FP8:
                    nc.tensor.matmul(
                        ot_pair[h],
                        v8[:, mp, 2 * p + h, :, 0 : DH + 1],
                        bank[:, mp, :, h * 512 : (h + 1) * 512],
                        start=(mp == 0), stop=(mp == MPAIR - 1),
                        perf_mode=DR,
                    )
                else:
                    for par in range(2):
                        nc.tensor.matmul(
                            ot_pair[h],
                            v8[:, 2 * mp + par, 2 * p + h, :],
                            bank[:, mp, par, h * 512 : (h + 1) * 512],
                            start=(mp == 0 and par == 0),
                            stop=(mp == MPAIR - 1 and par == 1),
                        )

        # trailing work for a finished unit u: copy O, rowsums -> rinv
        def emit_post(u, ot_pair):
            p, qc = u // NQC, u % NQC
            qs = slice(qc * 512, (qc + 1) * 512)
            for h in range(2):
                nc.scalar.copy(
                    out=OT[p][h * 64 : (h + 1) * 64, qs],
                    in_=ot_pair[h][0:64, :],
                )
                rs_row = small.tile(
                    [1, 512], f32, tag=f"rsrow{h}", name=f"rsr{u}_{h}"
                )
                nc.vector.tensor_copy(out=rs_row, in_=ot_pair[h][64:65, :])
                nc.sync.dma_start(
                    out=rs_dram[2 * p + h : 2 * p + h + 1, qs], in_=rs_row
                )
            rs_t = small.tile([128, 2, 4], f32, tag="rs", name=f"rs{u}")
            nc.sync.dma_start(
                out=rs_t,
                in_=rs_dram[2 * p : 2 * p + 2, qs].rearrange(
                    "h (p f) -> p h f", f=4
                ),
            )
            rinv_t = small.tile([128, 2, 4], f32, tag="ri", name=f"ri{u}")
            nc.vector.reciprocal(out=rinv_t, in_=rs_t)
            nc.sync.dma_start(
                out=rinv_dram[2 * p : 2 * p + 2, qs].rearrange(
                    "h (p f) -> p h f", f=4
                ),
                in_=rinv_t,
            )

        def emit_norm(u):
            p, qc = u // NQC, u % NQC
            qs = slice(qc * 512, (qc + 1) * 512)
            rbc = small.tile([128, 512], f32, tag="rbc", name=f"rbc{u}")
            for h in range(2):
                nc.sync.dma_start(
                    out=rbc[h * 64 : (h + 1) * 64, :],
                    in_=rinv_dram[
                        2 * p + h : 2 * p + h + 1, qs
                    ].to_broadcast([64, 512]),
                )
            nc.gpsimd.tensor_mul(OT[p][:, qs], OT[p][:, qs], rbc)

        def emit_y(u, t):
            p, qc = u // NQC, u % NQC
            qt = qc * 4 + t
            ys = ys_pool.tile([128, C], bf16, tag="ys", name=f"ys{u}_{t}")
            for ch in range(2):
                ps = ps_pp.tile([128, 512], f32, tag="pp", name=f"psy{u}{t}{ch}")
                nc.tensor.matmul(
                    ps,
                    OT[p][:, qt * 128 : (qt + 1) * 128],
                    Wout_t[:, p, ch * 512 : (ch + 1) * 512],
                    start=True, stop=True,
                )
                if t == 0:
                    nc.scalar.copy(
                        out=ys[:, ch * 512 : (ch + 1) * 512], in_=ps
                    )
                else:
                    nc.vector.tensor_copy(
                        out=ys[:, ch * 512 : (ch + 1) * 512], in_=ps
                    )
            nc.sync.dma_start(
                out=yp.ap()[p, qt * 128 : (qt + 1) * 128, :], in_=ys
            )

        # ---- weave schedules ----
        # proj emits inside unit u, popped between mt steps
        proj_sched = {
            0: [lambda: emit_q(0, 1), lambda: emit_k(0, 1),
                lambda: emit_k(0, 2), lambda: emit_k(0, 3)]
               + [lambda mt=mt: emit_v(mt) for mt in range(MT)],
            1: [lambda: emit_q(0, 2), lambda: emit_k(1, 0), lambda: emit_k(1, 1)],
            2: [lambda: emit_q(0, 3), lambda: emit_k(1, 2), lambda: emit_q(1, 0)],
            3: [lambda: emit_k(1, 3), lambda: emit_q(1, 1)],
            4: [lambda: emit_q(1, 2), lambda: emit_k(2, 0)],
            5: [lambda: emit_q(1, 3), lambda: emit_k(2, 1)],
            6: [lambda: emit_k(2, 2), lambda: emit_q(2, 0)],
            7: [lambda: emit_k(2, 3), lambda: emit_q(2, 1)],
            8: [lambda: emit_q(2, 2), lambda: emit_k(3, 0)],
            9: [lambda: emit_q(2, 3), lambda: emit_k(3, 1)],
            10: [lambda: emit_k(3, 2), lambda: emit_q(3, 0)],
            11: [lambda: emit_k(3, 3), lambda: emit_q(3, 1)],
            12: [lambda: emit_q(3, 2)],
            13: [lambda: emit_q(3, 3)],
        }

        # ---- bootstrap ----
        emit_q(0, 0)
        emit_k(0, 0)

        prev_ot = None       # ot pair of unit u-1 (for post)
        norm_u = None        # unit whose norm should be emitted
        y_u = None           # unit whose Y should be emitted

        for u in range(NU):
            queue = []
            if prev_ot is not None:
                up = u - 1
                queue.append(lambda up=up, po=prev_ot: emit_post(up, po))
            queue.extend(proj_sched.get(u, []))
            if norm_u is not None:
                queue.append(lambda nu=norm_u: emit_norm(nu))
            if y_u is not None and y_u <= 9:
                for t in range(4):
                    queue.append(lambda yu=y_u, t=t: emit_y(yu, t))

            ot_pair = [
                ps_ot.tile([65, 512], f32, tag=f"ot{h}", name=f"ot{u}_{h}")
                for h in range(2)
            ]
            nq = len(queue)
            popped = 0
            if u == 0:
                # AV(mp) legal only after emit_v(2mp+1) popped; queue has
                # 4 q/k emits then 16 V emits: V(j) is queue item 4+j.
                pending_av = list(range(MPAIR))
            for mt in range(MT):
                st = emit_s(u, mt)
                emit_exp(u, mt, st)
                # pop weave items: spread queue over the 16 mt steps
                want = (mt + 1) * nq // MT
                while popped < want:
                    queue[popped]()
                    popped += 1
                if u == 0:
                    while pending_av and 4 + 2 * pending_av[0] + 1 < popped:
                        emit_av(u, pending_av.pop(0), ot_pair)
                elif mt % 2 == 1 and mt >= 3:
                    emit_av(u, (mt - 3) // 2, ot_pair)
            while popped < nq:
                queue[popped]()
                popped += 1
            if u == 0:
                while pending_av:
                    emit_av(u, pending_av.pop(0), ot_pair)
            else:
                emit_av(u, MPAIR - 1, ot_pair)

            prev_ot = ot_pair
            norm_u = u - 1 if u >= 1 else None
            y_u = u - 2 if u >= 2 else None

        # ---- tail: flush remaining post/norm/Y; Y(12..13) fill the PE
        # while the last units' rowsum->rinv DMA chains complete ----
        emit_post(NU - 1, prev_ot)
        for t in range(4):
            emit_y(NU - 6, t)
        for t in range(4):
            emit_y(NU - 5, t)
        for t in range(4):
            emit_y(NU - 4, t)
        for t in range(4):
            emit_y(NU - 3, t)
        emit_norm(NU - 2)
        for t in range(4):
            emit_y(NU - 2, t)
        emit_norm(NU - 1)
        for t in range(4):
            emit_y(NU - 1, t)

    _split_excess_waits(nc)
    return nc


def make_in_maps(x, Wq, Wkv, Wout, bout):
    x = np.asarray(x, dtype=np.float32)
    Wq = np.asarray(Wq, dtype=np.float32)
    Wkv = np.asarray(Wkv, dtype=np.float32)
    Wout = np.asarray(Wout, dtype=np.float32)
    Wq_s = Wq * SCALE
    Wk = Wkv[:, :C]
    Wv = Wkv[:, C:]
    in_maps = []
    for core in range(NCORES):
        b, g = core // 2, core % 2
        cs = slice(g * 512, (g + 1) * 512)
        in_maps.append(
            dict(
                xT=np.ascontiguousarray(x[b].T).astype(_BF16),
                Wq=np.ascontiguousarray(Wq_s[:, cs]).astype(_BF16),
                Wk=np.ascontiguousarray(Wk[:, cs]).astype(_BF16),
                Wv=np.ascontiguousarray(Wv[:, cs]).astype(_BF16),
                Wout=np.ascontiguousarray(Wout[cs, :]).astype(_BF16),
            )
        )
    return in_maps


def assemble(results, bout):
    bout = np.asarray(bout, dtype=np.float32)
    out = np.empty((B, N, C), dtype=np.float32)
    for b in range(B):
        acc = results[2 * b]["yp"].astype(np.float32).sum(axis=0)
        acc += results[2 * b + 1]["yp"].astype(np.float32).sum(axis=0)
        out[b] = acc + bout
    return out


def kernel(x, Wq, Wkv, Wout, bout):
    from concourse.bass_utils import run_bass_kernel_spmd

    if "nc" not in _cache:
        _cache["nc"] = build_nc()
    in_maps = make_in_maps(x, Wq, Wkv, Wout, bout)
    res = run_bass_kernel_spmd(_cache["nc"], in_maps, core_ids=list(range(NCORES)))
    return assemble(res.results, bout)
